# revision 39
# baseline (speedup 1.0000x reference)
"""Causal self-attention block (QKV proj + RoPE + causal attention + o_proj)
on 8 Trainium2 NeuronCores — single fused launch.

Sharding: tensor-parallel over heads end-to-end (vLLM style). Core c owns
head indices 4c..4c+3 of BOTH batch elements: it computes their QKV
projections, RoPE, causal attention, and the o_proj PARTIAL product
W_o[:, 512c:512(c+1)] @ y[512c:512(c+1), :].  The host sums the 8 partial
outputs (the moral equivalent of the tensor-parallel AllReduce — on-device
collectives fail to load under this runtime).

Key device-side structure:
- fp16 GEMM operands everywhere (1 PE cycle/row; fp32 PSUM accumulation).
- q/k produced feature-major so attention scores need no transposes;
  v token-major for the PV matmul. Per-(head,batch) DRAM scratch tensors so
  phase-B loads only depend on the phase-A stores they actually read.
- Softmax is max-free and the 1/sqrt(d) scale rides the Exp activation's
  free scale parameter. The softmax denominator is NOT computed with
  per-chunk ones-vector matmuls (that would double PV cost on the PE):
  probs accumulate into one SBUF accumulator on the Vector engine (whose
  adds measure 3.5x faster than GpSimd's), then one 128->1 matmul per
  query tile reduces across partitions, a rank-1 matmul replicates back,
  and DVE inverts.
- Diagonal score/PV/exp work is column-restricted to the causally needed
  range (saves PE + ACT cycles); only a single shared 128x128 triangle
  mask remains, applied on the otherwise-idle GpSimd.
- attention outputs (normalized) are written straight into a persistent
  SBUF tile (ySB) that phase C consumes — y never round-trips DRAM.
"""

import numpy as np

import concourse.bass as bass
import concourse.tile as tile
from concourse import bacc, mybir
from concourse.bass_utils import run_bass_kernel_spmd

HIDDEN = 4096
N_HEADS = 32
HEAD_DIM = 128
B, S = 2, 2048
T = B * S
N_CORES = 8
HPC = N_HEADS // N_CORES  # 4 heads per core (both batches)
ROPE_BASE = 10000.0
SCALE = float(HEAD_DIM) ** -0.5

FP = mybir.dt.float16
F32 = mybir.dt.float32

M_B = 1024               # token block in QKV GEMM
KC = HIDDEN // 128       # 32 contraction chunks
TQ = 512                 # query tile in attention
EXPT = mybir.ActivationFunctionType.Exp


def build_nc(debug=False):
    nc = bacc.Bacc(num_devices=N_CORES, trn_type="TRN2")

    xT = nc.declare_dram_parameter("xT", [KC, 128, T], FP, isOutput=False)
    wqk = nc.declare_dram_parameter("wqk", [8, 128, KC, 128], FP, isOutput=False)
    wv = nc.declare_dram_parameter("wv", [KC, 128, 512], FP, isOutput=False)
    cost = nc.declare_dram_parameter("cost", [128, T], FP, isOutput=False)
    sint = nc.declare_dram_parameter("sint", [128, T], FP, isOutput=False)
    tri = nc.declare_dram_parameter("tri", [128, 128], FP, isOutput=False)
    onesc = nc.declare_dram_parameter("onesc", [128, 1], FP, isOutput=False)
    onesr = nc.declare_dram_parameter("onesr", [1, 128], FP, isOutput=False)
    wo = nc.declare_dram_parameter("wo", [32, 128, HPC, 128], FP,
                                   isOutput=False)
    out = nc.declare_dram_parameter("out", [32, 128, T], FP, isOutput=True)

    # per-(head,batch) scratch so phase-B loads depend only on the stores
    # they read (lets their DMA start at phase A's midpoint)
    qd = [[nc.dram_tensor(f"qd{h}_{bb}", [128, S], FP) for bb in range(B)]
          for h in range(HPC)]
    kd = [[nc.dram_tensor(f"kd{h}_{bb}", [128, S], FP) for bb in range(B)]
          for h in range(HPC)]
    vd = [nc.dram_tensor(f"vd{bb}", [S, HPC * 128], FP) for bb in range(B)]

    if debug:
        qdo = nc.declare_dram_parameter("qdo", [HPC, B, 128, S], FP,
                                        isOutput=True)
        kdo = nc.declare_dram_parameter("kdo", [HPC, B, 128, S], FP,
                                        isOutput=True)
        vdo = nc.declare_dram_parameter("vdo", [B, S, HPC * 128], FP,
                                        isOutput=True)
        ydo = nc.declare_dram_parameter("ydo", [128, HPC, T], F32,
                                        isOutput=True)

    from contextlib import ExitStack
    with nc.allow_low_precision(reason="fp16 operands; fp32 accumulation"), \
         tile.TileContext(nc) as tc, ExitStack() as prefstack:
        # prefetch pool outlives phase A's pools: head 0's phase-B tiles are
        # DMA'd from the middle of phase A (their stores are complete by
        # then, and mid-A queue position means they transfer under phase A's
        # compute cover). Single-buffered tags -> no ring-reuse waits that
        # could block the Sync dispatch queue.
        prefpool = prefstack.enter_context(tc.tile_pool(name="pref", bufs=1))
        pref = {}

        # ---------------- Phase A: QKV projection + RoPE ----------------
        # ySB is allocated AFTER phase A's pools close so wv can stay
        # SBUF-resident through phase A (loaded once, not once per block:
        # each dma_start costs ~630ns of serialized Sync-engine dispatch,
        # and the 256 per-block wv loads dominated that queue)
        with tc.tile_pool(name="xblk", bufs=2) as xpool, \
             tc.tile_pool(name="wq", bufs=2) as wqpool, \
             tc.tile_pool(name="wvp", bufs=1) as wvpool, \
             tc.tile_pool(name="rope", bufs=2) as rpool, \
             tc.tile_pool(name="ev", bufs=2) as evpool, \
             tc.tile_pool(name="vev", bufs=3) as vevpool, \
             tc.tile_pool(name="psA", bufs=1, space="PSUM") as psA:
            wvsb = wvpool.tile([128, KC, 512], FP)
            for mb in range(T // M_B):
                bb = mb // 2
                t0 = mb * M_B
                tl = t0 % S
                xb = xpool.tile([128, KC, M_B], FP)
                # mb0: chunked load so the first matmuls start after ~0.25MB,
                # with the one-time wv load interleaved. Later blocks arrive
                # under compute cover; 2 dispatches keep the Sync queue short.
                if mb == 0:
                    # kc-chunk boundaries for x and wv: small leading chunks
                    # so the first v matmuls fire almost immediately
                    xsplit = [0, 2, 4, 8, 12, 16, 20, 24, 28, 32]
                    wsplit = [0, 2, 4, 8, 16, 24, 32]
                    xi = wi = 0
                    while xi < len(xsplit) - 1 or wi < len(wsplit) - 1:
                        if xi < len(xsplit) - 1:
                            a, b = xsplit[xi], xsplit[xi + 1]
                            nc.sync.dma_start(
                                xb[:, a:b, :],
                                xT[a:b, :, t0:t0 + M_B]
                                .rearrange("kc p t -> p kc t"))
                            xi += 1
                        if wi < len(wsplit) - 1:
                            a, b = wsplit[wi], wsplit[wi + 1]
                            nc.sync.dma_start(
                                wvsb[:, a:b, :],
                                wv[a:b].rearrange("kc p f -> p kc f"))
                            wi += 1
                else:
                    for kq in range(2):
                        nc.sync.dma_start(
                            xb[:, 16 * kq:16 * (kq + 1), :],
                            xT[16 * kq:16 * (kq + 1), :, t0:t0 + M_B]
                            .rearrange("kc p t -> p kc t"))

                # v: token-major [tok 128, feat 512], accumulate over kc.
                for half in range(M_B // 512):
                    vps = [psA.tile([128, 512], F32, tag=f"vps{tt}",
                                    bufs=1, name=f"vps{tt}")
                           for tt in range(4)]
                    for kc in range(KC):
                        for tt in range(4):
                            nc.tensor.matmul(
                                vps[tt][:],
                                xb[:, kc, half * 512 + tt * 128:
                                   half * 512 + (tt + 1) * 128],
                                wvsb[:, kc, :],
                                start=(kc == 0), stop=(kc == KC - 1))
                    for tt in range(4):
                        vsb = vevpool.tile([128, 512], FP, tag="vev")
                        nc.scalar.copy(vsb[:], vps[tt][:])
                        r0 = tl + half * 512 + tt * 128
                        nc.sync.dma_start(vd[bb][r0:r0 + 128, :], vsb[:])

                # q, k: feature-major [head_dim 128, tok] + fused RoPE
                cq = rpool.tile([128, M_B], FP, tag="cos")
                sq = rpool.tile([128, M_B], FP, tag="sin")
                nc.sync.dma_start(cq[:], cost[:, t0:t0 + M_B])
                nc.sync.dma_start(sq[:], sint[:, t0:t0 + M_B])
                for ft in range(8):
                    qpa = psA.tile([128, 512], F32, tag="qkpsa", bufs=2)
                    qpb = psA.tile([128, 512], F32, tag="qkpsb", bufs=2)
                    wt = wqpool.tile([128, KC, 128], FP, tag="wqk", bufs=2)
                    nc.sync.dma_start(wt[:], wqk[ft])
                    for kc in range(KC):
                        nc.tensor.matmul(qpa[:], wt[:, kc, :],
                                         xb[:, kc, 0:512],
                                         start=(kc == 0), stop=(kc == KC - 1))
                        nc.tensor.matmul(qpb[:], wt[:, kc, :],
                                         xb[:, kc, 512:1024],
                                         start=(kc == 0), stop=(kc == KC - 1))
                    is_q = ft < 4
                    h = ft % 4
                    qsb = evpool.tile([128, M_B], FP, tag="qkev")
                    tmp = evpool.tile([128, M_B], FP, tag="ropetmp")
                    for hf, qps in ((0, qpa), (1, qpb)):
                        sl = slice(hf * 512, (hf + 1) * 512)
                        nc.vector.tensor_mul(qsb[:, sl], qps[:], cq[:, sl])
                        nc.vector.tensor_mul(tmp[0:64, sl], qps[64:128, :],
                                             sq[0:64, sl])
                        nc.vector.tensor_mul(tmp[64:128, sl], qps[0:64, :],
                                             sq[64:128, sl])
                        nc.vector.tensor_sub(qsb[0:64, sl], qsb[0:64, sl],
                                             tmp[0:64, sl])
                        nc.vector.tensor_add(qsb[64:128, sl],
                                             qsb[64:128, sl],
                                             tmp[64:128, sl])
                    dst = qd if is_q else kd
                    nc.sync.dma_start(dst[h][bb][:, tl:tl + M_B], qsb[:])

                if mb == 1:
                    # batch 0 of q/k/v fully stored: prefetch head 0's
                    # phase-B tiles, overlapping phase A's second half
                    pq = prefpool.tile([128, S], FP, tag="pq")
                    pk = prefpool.tile([128, S], FP, tag="pk")
                    pvt = prefpool.tile([128, S // 128, 128], FP, tag="pv")
                    nc.sync.dma_start(pq[:], qd[0][0][:])
                    nc.sync.dma_start(pk[:], kd[0][0][:])
                    nc.sync.dma_start(
                        pvt[:], vd[0][:, 0:128]
                        .rearrange("(c p) d -> p c d", p=128))
                    pref[(0, 0)] = (pq, pk, pvt)

        with tc.tile_pool(name="ypers", bufs=1) as ypers:
            ySB = ypers.tile([128, HPC, T], FP)

            # ---------------- Phase B: causal attention ----------------
            with tc.tile_pool(name="const", bufs=1) as cpool, \
                 tc.tile_pool(name="heads", bufs=3) as hpool, \
                 tc.tile_pool(name="probs", bufs=6) as ppool, \
                 tc.tile_pool(name="accp", bufs=2) as apool, \
                 tc.tile_pool(name="yt", bufs=2) as ypool, \
                 tc.tile_pool(name="psB", bufs=1, space="PSUM") as psB:
                ones_col = cpool.tile([128, 1], FP)
                nc.sync.dma_start(ones_col[:], onesc[:])
                ones_row = cpool.tile([1, 128], FP)
                nc.sync.dma_start(ones_row[:], onesr[:])
                trim = cpool.tile([128, 128], FP)
                nc.sync.dma_start(trim[:], tri[:])

                pending = [None]  # deferred per-(j) normalization closure

                for bb in range(B):
                    for h in range(HPC):
                        if (bb, h) in pref:
                            qh, kh, vh = pref[(bb, h)]
                        else:
                            qh = hpool.tile([128, S], FP, tag="qh")
                            kh = hpool.tile([128, S], FP, tag="kh")
                            vh = hpool.tile([128, S // 128, 128], FP,
                                            tag="vh")
                            nc.sync.dma_start(qh[:], qd[h][bb][:])
                            nc.sync.dma_start(kh[:], kd[h][bb][:])
                            nc.sync.dma_start(
                                vh[:], vd[bb][:, h * 128:(h + 1) * 128]
                                .rearrange("(c p) d -> p c d", p=128))
                        for j in range(S // TQ):
                            npairs = 2 * j + 2
                            nchunks = 4 * j + 4
                            acc_v = apool.tile([128, TQ], FP, tag="accv")
                            nc.gpsimd.memset(acc_v[:], 0.0)
                            aps = psB.tile([128, TQ], F32, tag="aps", bufs=2)

                            def scores(p, j=j, qh=qh, kh=kh):
                                # pair-level column restriction for diagonal
                                dd = p - 2 * j
                                poff = 256 * dd if dd > 0 else 0
                                sps = psB.tile([128, 2, TQ], F32, tag="sps",
                                               bufs=2, name="sps")
                                for hf in range(2):
                                    c = 2 * p + hf
                                    nc.tensor.matmul(
                                        sps[:, hf, poff:],
                                        kh[:, c * 128:(c + 1) * 128],
                                        qh[:, j * TQ + poff:(j + 1) * TQ],
                                        start=True, stop=True)
                                pr = ppool.tile([128, 2, TQ], FP, tag="pr",
                                                name="pr")
                                nc.scalar.activation(pr[:, :, poff:],
                                                     sps[:, :, poff:], EXPT,
                                                     scale=SCALE)
                                return pr

                            def mask_acc(p, pr, j=j, acc_v=acc_v):
                                dd = p - 2 * j
                                if dd >= 0:
                                    # diagonal pair: triangle masks on gpsimd,
                                    # restricted accumulate on DVE
                                    for hf in range(2):
                                        o = 128 * (2 * dd + hf)
                                        nc.gpsimd.tensor_mul(
                                            pr[:, hf, o:o + 128],
                                            pr[:, hf, o:o + 128], trim[:])
                                    for hf in range(2):
                                        o = 128 * (2 * dd + hf)
                                        nc.vector.tensor_add(
                                            acc_v[:, o:], acc_v[:, o:],
                                            pr[:, hf, o:])
                                else:
                                    # all accumulation on DVE: its adds are
                                    # 3.5x faster than GpSimd's (measured),
                                    # and an empty GpSimd queue keeps the
                                    # mask->PV dependency chain short
                                    for hf in range(2):
                                        nc.vector.tensor_add(acc_v[:],
                                                             acc_v[:],
                                                             pr[:, hf, :])

                            def pv(p, pr, j=j, aps=aps, vh=vh,
                                   nchunks=nchunks):
                                for hf in range(2):
                                    c = 2 * p + hf
                                    d = c - 4 * j
                                    o = 128 * d if d > 0 else 0
                                    nc.tensor.matmul(
                                        aps[:, o:], vh[:, c, :],
                                        pr[:, hf, o:],
                                        start=(c == 0), stop=(c == nchunks - 1),
                                        skip_group_check=True)

                            prev = scores(0)
                            if pending[0] is not None:
                                pending[0]()
                                pending[0] = None
                            mask_acc(0, prev)
                            for p in range(1, npairs):
                                cur = scores(p)
                                mask_acc(p, cur)
                                pv(p - 1, prev)
                                prev = cur
                            pv(npairs - 1, prev)

                            lps = psB.tile([1, TQ], F32, tag="lps", bufs=1)
                            nc.tensor.matmul(lps[:], ones_col[:], acc_v[:],
                                             start=True, stop=True,
                                             skip_group_check=True)

                            def fin(lps=lps, aps=aps, bb=bb, h=h, j=j):
                                r = ypool.tile([1, TQ], FP, tag="r")
                                nc.vector.tensor_copy(r[:], lps[:])
                                rps = psB.tile([128, TQ], F32, tag="rps",
                                               bufs=1)
                                nc.tensor.matmul(rps[:], ones_row[:], r[:],
                                                 start=True, stop=True)
                                rinv = ypool.tile([128, TQ], F32, tag="rinv")
                                nc.vector.reciprocal_approx_fast(rinv[:],
                                                                 rps[:])
                                q0 = bb * S + j * TQ
                                nc.vector.tensor_mul(ySB[:, h, q0:q0 + TQ],
                                                     aps[:], rinv[:])
                            pending[0] = fin
                pending[0]()
                pending[0] = None

            # ---------------- Phase C: o_proj partial ----------------
            with tc.tile_pool(name="wop", bufs=3) as wopool, \
                 tc.tile_pool(name="oev", bufs=4) as oepool, \
                 tc.tile_pool(name="psD", bufs=1, space="PSUM") as psD:
                for of in range(32):
                    wt = wopool.tile([128, HPC, 128], FP, tag="wo")
                    nc.sync.dma_start(wt[:], wo[of])
                    # each stationary weight block feeds 4 matmuls (4 PSUM
                    # tiles x bufs=2 = all 8 banks) so LDWEIGHTS hides fully;
                    # the 4 evacuations batch into one [128, 4, 512] SBUF
                    # tile per 2048-token store (1 DMA dispatch instead of 4)
                    for g in range(2):
                        osb = oepool.tile([128, 4, 512], FP, tag="oev")
                        ops = [psD.tile([128, 512], F32, tag=f"ops{q}",
                                        bufs=2, name=f"ops{q}")
                               for q in range(4)]
                        t0 = g * 2048
                        for kc in range(HPC):
                            for q in range(4):
                                nc.tensor.matmul(
                                    ops[q][:], wt[:, kc, :],
                                    ySB[:, kc,
                                        t0 + q * 512:t0 + (q + 1) * 512],
                                    start=(kc == 0), stop=(kc == HPC - 1))
                        # evacuate on both Scalar and Vector (different
                        # PSUM banks -> legal in parallel; DVE is idle in
                        # this phase) so bank release never gates the next
                        # weight block's matmuls
                        for q in range(4):
                            if q % 2 == 0:
                                nc.scalar.copy(osb[:, q, :], ops[q][:])
                            else:
                                nc.vector.tensor_copy(osb[:, q, :],
                                                      ops[q][:])
                        nc.sync.dma_start(
                            out[of][:, g * 2048:(g + 1) * 2048]
                            .rearrange("p (c f) -> p c f", c=4), osb[:])

            if debug:
                with tc.tile_pool(name="dbg", bufs=2) as dpool:
                    for h in range(HPC):
                        for bb in range(B):
                            nc.sync.dma_start(qdo[h, bb], qd[h][bb][:])
                            nc.sync.dma_start(kdo[h, bb], kd[h][bb][:])
                    for bb in range(B):
                        nc.sync.dma_start(vdo[bb], vd[bb][:])
                    for h in range(HPC):
                        yb = dpool.tile([128, T], F32, tag="yb")
                        nc.vector.tensor_copy(yb[:], ySB[:, h, :])
                        nc.sync.dma_start(ydo[:, h, :], yb[:])

    nc.finalize()
    return nc


_NC = None


def get_nc():
    global _NC
    if _NC is None:
        _NC = build_nc()
    return _NC


def _rope_tables(positions):
    """positions [B, S] int -> cos/sin tables [128, T] fp16 in token order
    (b*S + t); rows [0:64] and [64:128] hold the same 64 freqs. The 1/sqrt(d)
    score scale is applied later inside the Exp activation, not here."""
    inv_freq = 1.0 / (ROPE_BASE ** (np.arange(0, HEAD_DIM, 2, dtype=np.float64)
                                    / HEAD_DIM))
    freqs = np.asarray(positions).reshape(T).astype(np.float64)[:, None] * inv_freq
    cos = np.cos(freqs).T.astype(np.float32)  # [64, T]
    sin = np.sin(freqs).T.astype(np.float32)
    cos2 = np.concatenate([cos, cos], axis=0).astype(np.float16)
    sin2 = np.concatenate([sin, sin], axis=0).astype(np.float16)
    return cos2, sin2


def prepare_inputs(hidden_states, positions, W_pack, W_o):
    x = np.ascontiguousarray(np.asarray(hidden_states, dtype=np.float32)
                             .reshape(T, HIDDEN))
    xT_blocks = np.ascontiguousarray(x.T.astype(np.float16)).reshape(KC, 128, T)

    cos2, sin2 = _rope_tables(positions)

    p = np.arange(128)[:, None]
    q = np.arange(128)[None, :]
    tri = (p <= q).astype(np.float16)

    W_pack = np.asarray(W_pack, dtype=np.float32)
    W_o = np.asarray(W_o, dtype=np.float32)

    in_maps = []
    for c in range(N_CORES):
        hs = [HPC * c + i for i in range(HPC)]
        wqk_blocks = np.empty((8, 128, KC, 128), dtype=np.float16)
        for ft in range(8):
            off = 0 if ft < 4 else HIDDEN
            h = hs[ft % 4]
            wsl = W_pack[off + h * 128: off + (h + 1) * 128, :]  # [128, 4096]
            wqk_blocks[ft] = wsl.reshape(128, KC, 128).transpose(2, 1, 0)
        wv_sl = np.concatenate(
            [W_pack[2 * HIDDEN + h * 128: 2 * HIDDEN + (h + 1) * 128, :]
             for h in hs], axis=0)  # [512, 4096]
        wv_blocks = np.ascontiguousarray(
            wv_sl.astype(np.float16).reshape(512, KC, 128).transpose(1, 2, 0))
        wo_c = W_o[:, 512 * c: 512 * (c + 1)].astype(np.float16)  # [4096, 512]
        # param layout (of, p, kc, f): wo[of, p, kc, f] = W_o[of*128+f,
        # 512c + kc*128 + p] -- matches the [128, HPC, 128] SBUF tile walk
        wo_blocks = np.ascontiguousarray(
            wo_c.reshape(32, 128, HPC, 128).transpose(0, 3, 2, 1))
        in_maps.append({
            "xT": xT_blocks,
            "wqk": np.ascontiguousarray(wqk_blocks),
            "wv": wv_blocks,
            "cost": cos2, "sint": sin2,
            "tri": tri,
            "onesc": np.ones((128, 1), dtype=np.float16),
            "onesr": np.ones((1, 128), dtype=np.float16),
            "wo": wo_blocks,
        })
    return in_maps


def finalize(results):
    acc = np.zeros((HIDDEN, T), dtype=np.float32)
    for c in range(N_CORES):
        acc += results[c]["out"].astype(np.float32).reshape(HIDDEN, T)
    return np.ascontiguousarray(acc.T).reshape(B, S, HIDDEN)


def kernel(hidden_states, positions, W_pack, W_o):
    nc = get_nc()
    in_maps = prepare_inputs(hidden_states, positions, W_pack, W_o)
    res = run_bass_kernel_spmd(nc, in_maps, list(range(N_CORES)))
    return finalize(res.results)
